# revision 1
# baseline (speedup 1.0000x reference)
"""Locally-connected layer (3x3, stride 1) on 8 TRN2 NeuronCores.

Math (per reference): out[b,o,i,j] = sum_{c,kh,kw} x[b,c,i+kh,j+kw] * W[c,o,i,j,kh,kw] + bias[o,i,j]
  x: [128, 64, 32, 32] f32, W: [64, 64, 30, 30, 3, 3] f32, bias: [64, 30, 30] f32
  out: [128, 64, 30, 30] f32

Sharding: each core owns 4 output rows (cores 6,7 overlap rows 24-27/26-29 so all
cores run an identical program; host keeps rows 28-29 from core 7).

Per-core kernel: for each output position (i,j) accumulate 9 matmuls
  psum[b=128, o=64] += xT[c, b]^T @ W[c, o]  over taps (kh, kw)
with the x-pixel tile [c=64(+ones), b=128] as the PE-stationary operand shared
across all taps/rows that read pixel (h, w). Bias is added via a K=1 matmul of
an all-ones stationary row against the bias row, which also primes the PSUM
bank's has_written bits (start=True) for the whole bank.

Inputs are cast to bf16 and relaid on host so every device DMA is contiguous.
"""

import sys

for _p in ("/opt/trn_rl_repo",):
    if _p not in sys.path:
        sys.path.insert(0, _p)

import numpy as np
import ml_dtypes

import concourse.bass as bass
import concourse.tile as tile
from concourse import bacc, mybir
from concourse.bass_utils import run_bass_kernel_spmd

N_CORES = 8
B = 128
C = 64          # contracted channel dim (weight axis 0)
O = 64          # output channel dim (weight axis 1)
H = 32
W = 32
K = 3
OH = 30
OW = 30
R = 4           # output rows per core
H6 = R + K - 1  # input rows per core
ROW0 = [0, 4, 8, 12, 16, 20, 24, 26]  # first output row per core
BANDS = [(0, 8), (8, 8), (16, 8), (24, 6)]  # (j0, width) PSUM j-bands

XT_FREE = H6 * W * B            # xt free size: (h, w, b) = 24576
WT_PER_J = R * K * K * O        # 2304: (i, kh, kw, o)
WT_FREE = OW * WT_PER_J         # 69120

_BF16 = ml_dtypes.bfloat16


def build_nc(repeat: int = 1, mode: str = "full"):
    """Build the per-core Bass program. `repeat` wraps the compute in a
    hardware loop (used only for timing)."""
    nc = bacc.Bacc("TRN2", target_bir_lowering=False, debug=False,
                   num_devices=N_CORES)
    xt_ap = nc.dram_tensor("xt", [C + 1, XT_FREE], mybir.dt.bfloat16,
                           kind="ExternalInput").ap()
    wt_ap = nc.dram_tensor("wt", [C + 1, WT_FREE], mybir.dt.bfloat16,
                           kind="ExternalInput").ap()
    out_ap = nc.dram_tensor("outp", [R, OW, B, O], mybir.dt.float32,
                            kind="ExternalOutput").ap()

    with tile.TileContext(nc) as tc:
        with (
            tc.tile_pool(name="xpool", bufs=1) as xpool,
            tc.tile_pool(name="wpool", bufs=2) as wpool,
            tc.tile_pool(name="ppool", bufs=8, space="PSUM") as ppool,
            tc.tile_pool(name="opool", bufs=4) as opool,
        ):
            xt_sb = xpool.tile([C + 1, XT_FREE], mybir.dt.bfloat16)
            # 8 chunked DMAs (split on h*w) to spread across queues
            n_x_dma = 8
            xchunk = XT_FREE // n_x_dma
            for q in range(n_x_dma):
                nc.sync.dma_start(xt_sb[:, q * xchunk:(q + 1) * xchunk],
                                  xt_ap[:, q * xchunk:(q + 1) * xchunk])
            # 3D views: [part, (h,w), b]
            xt3 = xt_sb[:].rearrange("p (f b) -> p f b", b=B)

            def body():
                if mode == "empty":
                    return
                for (j0, bw) in BANDS:
                    wt_sb = wpool.tile([C + 1, BANDS[0][1] * WT_PER_J],
                                       mybir.dt.bfloat16, tag="wt")
                    half = bw * WT_PER_J // 2
                    nc.sync.dma_start(wt_sb[:, :half],
                                      wt_ap[:, j0 * WT_PER_J:j0 * WT_PER_J + half])
                    nc.sync.dma_start(wt_sb[:, half:bw * WT_PER_J],
                                      wt_ap[:, j0 * WT_PER_J + half:(j0 + bw) * WT_PER_J])
                    wt3 = wt_sb[:].rearrange("p (j r) -> p j r", r=WT_PER_J)

                    ps = [ppool.tile([B, 512], mybir.dt.float32, tag="ps",
                                     name=f"ps{i}")
                          for i in range(R)]
                    # bias + has_written priming: ones[1,128]^T @ bias[1, bw*64]
                    ones_ap = xt3[C:C + 1, 0, :]
                    for i in range(R):
                        bias_rhs = wt3[C:C + 1, 0:bw, i * K * K * O:i * K * K * O + O]
                        nc.tensor.matmul(ps[i][:, 0:bw * O], ones_ap, bias_rhs,
                                         start=True, stop=(mode == "nomm"))
                    if mode != "nomm":
                        for h in range(H6):
                            for w in range(j0, min(j0 + bw + 2, W)):
                                lhs = xt3[0:C, h * W + w, :]
                                for kh in range(K):
                                    i = h - kh
                                    if i < 0 or i >= R:
                                        continue
                                    for kw in range(K):
                                        j = w - kw
                                        if j < j0 or j >= j0 + bw:
                                            continue
                                        jl = j - j0
                                        off = jl * WT_PER_J + i * (K * K * O) \
                                            + kh * (K * O) + kw * O
                                        is_last = (kh == K - 1 and kw == K - 1
                                                   and j == j0 + bw - 1)
                                        nc.tensor.matmul(
                                            ps[i][:, jl * O:(jl + 1) * O],
                                            lhs,
                                            wt_sb[0:C, off:off + O],
                                            start=False, stop=is_last)
                    for i in range(R):
                        ob = opool.tile([B, BANDS[0][1] * O], mybir.dt.float32,
                                        tag="ob")
                        nc.vector.tensor_copy(ob[:, 0:bw * O], ps[i][:, 0:bw * O])
                        dst = out_ap[i, j0:j0 + bw].transpose([1, 0, 2])
                        nc.sync.dma_start(
                            dst, ob[:, 0:bw * O].rearrange("p (j o) -> p j o", o=O))

            if repeat == 1:
                body()
            else:
                with tc.For_i(0, repeat, 1):
                    body()

    nc.compile()
    dedup_ldweights(nc)
    return nc


def dedup_ldweights(nc):
    """Remove consecutive InstLdweights with identical weight APs from the PE
    stream (post-compile). The PE array keeps the stationary operand loaded
    across matmuls, so a reload of the same AP is pure overhead (~50-110ns
    each). Conservative: keeps any LDW that carries sync waits/updates or
    follows an intervening different LDW."""
    removed = 0
    for blk in nc.m.functions[0].blocks:
        insts = list(blk.instructions)
        has_pe = any(type(i).__name__ == "InstLdweights" for i in insts)
        if not has_pe:
            continue
        prev_key = None
        to_remove = []
        for inst in insts:
            nm = type(inst).__name__
            if nm == "InstLdweights":
                key = repr(inst.ins[0])
                si = inst.sync_info
                clean = not si or (not si.on_wait and not si.on_update)
                if key == prev_key and clean:
                    to_remove.append(inst)
                else:
                    prev_key = key
            elif nm == "InstMatmult":
                pass  # matmuls don't disturb the loaded stationary
            elif nm in ("InstEventSemaphore", "InstNop", "InstTensorLoad",
                        "InstTensorSave"):
                pass  # sequencer-only ops don't touch the PE array
            else:
                prev_key = None  # unknown PE-array effect: be safe
        for inst in to_remove:
            blk.instructions.remove(inst)
            removed += 1
    return removed


def prep_inputs(x: np.ndarray, weight: np.ndarray, bias: np.ndarray):
    """Host-side shard + relayout + bf16 cast. Returns in_maps for 8 cores."""
    in_maps = []
    for r0 in ROW0:
        # xt[c, h, w, b] (+ ones row) -> [65, 24576]
        xs = x[:, :, r0:r0 + H6, :]                      # [B, C, H6, W]
        xt = np.empty((C + 1, H6, W, B), dtype=_BF16)
        xt[:C] = xs.transpose(1, 2, 3, 0).astype(_BF16)
        xt[C] = np.ones((H6, W, B), dtype=_BF16)
        # wt[c, j, i, kh, kw, o] (+ bias row) -> [65, 69120]
        ws = weight[:, :, r0:r0 + R, :, :, :]            # [C, O, R, OW, K, K]
        wt = np.empty((C + 1, OW, R, K, K, O), dtype=_BF16)
        wt[:C] = ws.transpose(0, 3, 2, 4, 5, 1).astype(_BF16)
        wt[C] = 0
        wt[C, :, :, 0, 0, :] = bias[:, r0:r0 + R, :].transpose(2, 1, 0).astype(_BF16)
        in_maps.append({
            "xt": np.ascontiguousarray(xt.reshape(C + 1, XT_FREE)),
            "wt": np.ascontiguousarray(wt.reshape(C + 1, WT_FREE)),
        })
    return in_maps


def gather_output(results):
    out = np.empty((B, O, OH, OW), dtype=np.float32)
    for k, r0 in enumerate(ROW0):
        co = results[k]["outp"]                           # [R, OW, B, O]
        lo = 0 if k < 7 else 2                            # core 7: keep rows 28-29
        out[:, :, r0 + lo:r0 + R, :] = co[lo:].transpose(2, 3, 0, 1)
    return out


_NC_CACHE = {}


def kernel(x: np.ndarray, weight: np.ndarray, bias: np.ndarray) -> np.ndarray:
    if "nc" not in _NC_CACHE:
        _NC_CACHE["nc"] = build_nc()
    nc = _NC_CACHE["nc"]
    in_maps = prep_inputs(np.asarray(x), np.asarray(weight), np.asarray(bias))
    res = run_bass_kernel_spmd(nc, in_maps, core_ids=list(range(N_CORES)))
    return gather_output(res.results)



# revision 10
# speedup vs baseline: 2.1042x; 2.1042x over previous
"""Locally-connected layer (3x3, stride 1) on 8 TRN2 NeuronCores — v2.

Math: out[b,o,i,j] = sum_{c,kh,kw} x[b,c,i+kh,j+kw] * W[c,o,i,j,kh,kw] + bias[o,i,j]
  x: [128, 64, 32, 32] f32, W: [64, 64, 30, 30, 3, 3] f32, bias: [64, 30, 30] f32
  out: [128, 64, 30, 30] f32

Sharding: each core owns 4 output rows (cores 6,7 overlap rows so all cores run
an identical program; host keeps rows 28-29 from core 7).

Per-core schedule (data bf16, PSUM f32):
- Contract (c, kh-pair): SBUF x tile packs channels of pixel rows (h, h+1) on
  partitions (0-63, 64-127).  A "pair" matmul contracts kh=(0,1) for output row
  i=h in one 128-deep pass; kh=2 is a 64-deep "single" matmul against one half
  of the same loaded stationary (row-group select via base partition).
- Diagonal weight layout: for stationary x-column w, one matmul covers output
  columns j=w-2..w (kw=2-t), N up to 192.  Matmuls split at PSUM bank edges.
- PSUM: one bank per (row, j-band of 8).  start=True on the first matmul into a
  bank clears its has_written bits; per-element first touch then overwrites.
- Eviction: DVE (psum + bias) -> bf16 out staging; 2 contiguous out DMAs.
  Host upcasts to f32.

Sweep order (h = x slab, pairs (h,h+1)): h=0 pair(i=0); h=1 pair(i=1) +
single(i=0, bottom); h=2 pair(i=2); h=3 singles(i=1 top, i=2 bottom);
h=3 pair(i=3); h=5 single(i=3, top).  Keeps <= 8 PSUM banks live.
"""

import sys

for _p in ("/opt/trn_rl_repo",):
    if _p not in sys.path:
        sys.path.insert(0, _p)

import numpy as np
import ml_dtypes

import concourse.bass as bass
import concourse.tile as tile
from concourse import bacc, mybir
from concourse.bass_utils import run_bass_kernel_spmd

N_CORES = 8
B = 128
C = 64
O = 64
H = 32
W = 32
K = 3
OH = 30
OW = 30
R = 4
ROW0 = [0, 4, 8, 12, 16, 20, 24, 26]

NSLAB = 5                      # x slabs at h = 0,1,2,3,5 (pair = rows (h, h+1))
SLABS = [0, 1, 2, 3, 5]
XT_FREE = NSLAB * W * B        # 20480 elems / partition
WCH = W * K * O                # 6144: one weight chunk (w, t, o)
OUT_FREE = R * OW * O          # 7680

_BF16 = ml_dtypes.bfloat16


def _bank_split(j_lo, j_hi):
    """Split [j_lo, j_hi] into (a, b, bank) within-bank runs."""
    segs = []
    a = j_lo
    while a <= j_hi:
        bk = a // 8
        b = min(j_hi, bk * 8 + 7)
        segs.append((a, b, bk))
        a = b + 1
    return segs


def _segments(w, kind):
    """(a, b, bank, fresh) matmul output runs for stationary column w.

    A pair sweep writes column j=w for the first time (fresh: overwrite path,
    start=True at bank starts) while j=w-2..w-1 accumulate; each matmul's
    region must be homogeneous, so fresh and old are separate matmuls.
    Single sweeps always accumulate (the row's pair sweep ran first)."""
    j_lo, j_hi = max(0, w - 2), min(OH - 1, w)
    if j_lo > j_hi:
        return []
    if kind == "pair":
        segs = []
        if w >= 1:
            segs += [(a, b, bk, False) for (a, b, bk)
                     in _bank_split(j_lo, min(OH - 1, w - 1))]
        if w <= OH - 1:
            segs.append((w, w, w // 8, True))
        return segs
    return [(a, b, bk, False) for (a, b, bk) in _bank_split(j_lo, j_hi)]


def build_nc(repeat: int = 1, mode: str = "full"):
    nc = bacc.Bacc("TRN2", target_bir_lowering=False, debug=False,
                   num_devices=N_CORES)
    xt_ap = nc.dram_tensor("xt", [B, XT_FREE], mybir.dt.bfloat16,
                           kind="ExternalInput").ap()
    wp_ap = nc.dram_tensor("wp", [B, R * WCH], mybir.dt.bfloat16,
                           kind="ExternalInput").ap()
    ws_ap = nc.dram_tensor("ws", [B, 2 * WCH], mybir.dt.bfloat16,
                           kind="ExternalInput").ap()
    bias_ap = nc.dram_tensor("biasb", [B, OUT_FREE], mybir.dt.float32,
                             kind="ExternalInput").ap()
    out_ap = nc.dram_tensor("outp", [B, OUT_FREE], mybir.dt.bfloat16,
                            kind="ExternalOutput").ap()

    with tile.TileContext(nc) as tc:
        with (
            tc.tile_pool(name="xpool", bufs=1) as xpool,
            tc.tile_pool(name="bpool", bufs=1) as bpool,
            tc.tile_pool(name="wppool", bufs=4) as wppool,
            tc.tile_pool(name="wspool", bufs=2) as wspool,
            tc.tile_pool(name="ppool", bufs=8, space="PSUM") as ppool,
            tc.tile_pool(name="opool", bufs=2) as opool,
        ):
            xt_sb = xpool.tile([B, XT_FREE], mybir.dt.bfloat16)
            nc.scalar.dma_start(xt_sb[:, :XT_FREE // 2], xt_ap[:, :XT_FREE // 2])
            nc.scalar.dma_start(xt_sb[:, XT_FREE // 2:], xt_ap[:, XT_FREE // 2:])
            bias_sb = bpool.tile([B, OUT_FREE], mybir.dt.float32)
            nc.scalar.dma_start(bias_sb, bias_ap)
            x4 = xt_sb[:].rearrange("p (h w b) -> p h w b", w=W, b=B)

            FULL, TOP, BOT = (0, B), (0, C), (C, B)

            def body():
                if mode == "empty":
                    return
                wp, ws = {}, {}

                def load_wp(i):
                    t = wppool.tile([B, WCH], mybir.dt.bfloat16, tag="wp")
                    nc.sync.dma_start(t, wp_ap[:, i * WCH:(i + 1) * WCH])
                    wp[i] = t[:].rearrange("p (w t o) -> p w t o", t=K, o=O)

                def load_ws(s):
                    t = wspool.tile([B, WCH], mybir.dt.bfloat16, tag="ws")
                    nc.sync.dma_start(t, ws_ap[:, s * WCH:(s + 1) * WCH])
                    ws[s] = t[:].rearrange("p (w t o) -> p w t o", t=K, o=O)

                load_wp(0)
                load_wp(1)
                load_ws(0)
                load_wp(2)
                load_ws(1)
                load_wp(3)

                ps = {}
                out_sb = opool.tile([B, OUT_FREE], mybir.dt.bfloat16)

                def evict(i, bk):
                    off = i * (OW * O) + bk * 8 * O
                    n = (min(OH, bk * 8 + 8) - bk * 8) * O
                    nc.vector.scalar_tensor_tensor(
                        out_sb[:, off:off + n], ps[(i, bk)][:, :n], 1.0,
                        bias_sb[:, off:off + n],
                        op0=mybir.AluOpType.mult, op1=mybir.AluOpType.add)
                    ps.pop((i, bk))

                def sweep(slab, entries):
                    """entries: list of (kind, i, (lo, hi), w-view); emitted
                    interleaved per w so they share the loaded stationary."""
                    for w in range(W):
                        for (kind, i, (lo, hi), wv) in entries:
                            lhsT = x4[lo:hi, slab, w, :]
                            for (a, b, bk, fresh) in _segments(w, kind):
                                t0 = a - (w - 2)
                                nt = b - a + 1
                                rhs = wv[lo:hi, w, t0:t0 + nt, :]
                                start = fresh and w == 8 * bk
                                if start:
                                    ps[(i, bk)] = ppool.tile(
                                        [B, 512], mybir.dt.float32, tag="ps",
                                        name=f"ps{i}_{bk}")
                                stop = (kind == "single"
                                        and ((bk < 3 and w == 8 * bk + 9)
                                             or (bk == 3 and w == W - 1)))
                                dst = ps[(i, bk)][:, (a - 8 * bk) * O:
                                                  (b + 1 - 8 * bk) * O]
                                nc.tensor.matmul(dst, lhsT, rhs,
                                                 start=start, stop=stop)
                                if stop:
                                    evict(i, bk)

                if mode != "nomm":
                    sweep(0, [("pair", 0, FULL, wp[0])])
                    sweep(1, [("pair", 1, FULL, wp[1]),
                              ("single", 0, BOT, ws[0])])
                    sweep(2, [("pair", 2, FULL, wp[2])])
                    sweep(3, [("single", 1, TOP, ws[0]),
                              ("single", 2, BOT, ws[1])])
                    sweep(3, [("pair", 3, FULL, wp[3])])
                    sweep(4, [("single", 3, TOP, ws[1])])

                nc.scalar.dma_start(out_ap[:, :OUT_FREE // 2],
                                    out_sb[:, :OUT_FREE // 2])
                nc.scalar.dma_start(out_ap[:, OUT_FREE // 2:],
                                    out_sb[:, OUT_FREE // 2:])

            if repeat == 1:
                body()
            else:
                with tc.For_i(0, repeat, 1,
                              hint_engines=(mybir.EngineType.PE,)):
                    body()

    nc.compile()
    dedup_ldweights(nc)
    return nc


def _ldw_key(inst):
    """(repr of weights AP) for identity comparison."""
    return repr(inst.ins[0])


def dedup_ldweights(nc):
    """Remove InstLdweights that reload rows already holding the same data.

    Case 1: identical AP as the previous LdW (classic dedup).
    Case 2: the previous LdW loaded the full 128-partition stationary from the
    same tensor/offset, and this LdW loads a sub-partition-range of it with an
    identical free pattern — the target rows already hold those values.
    Conservative: any LdW carrying sync waits/updates is kept; any intervening
    non-PE-array instruction resets tracking.
    """
    removed = 0
    for blk in nc.m.functions[0].blocks:
        insts = list(blk.instructions)
        if not any(type(i).__name__ == "InstLdweights" for i in insts):
            continue
        prev_key = None
        to_remove = []
        for inst in insts:
            nm = type(inst).__name__
            if nm == "InstLdweights":
                key = _ldw_key(inst)
                si = inst.sync_info
                clean = not si or (not si.on_wait and not si.on_update)
                if key == prev_key and clean:
                    to_remove.append(inst)
                else:
                    prev_key = key
            elif nm == "InstMatmult":
                pass
            elif nm in ("InstEventSemaphore", "InstNop", "InstTensorLoad",
                        "InstTensorSave"):
                pass
            else:
                prev_key = None
        for inst in to_remove:
            blk.instructions.remove(inst)
            removed += 1
    return removed


def prep_inputs(x, weight, bias):
    """Host-side shard + relayout + bf16 cast. Returns in_maps for 8 cores."""
    x = np.asarray(x, dtype=np.float32)
    weight = np.asarray(weight, dtype=np.float32)
    bias = np.asarray(bias, dtype=np.float32)

    # (w, t) -> j / kw maps for the diagonal layout
    jm = (np.arange(W)[:, None] - 2) + np.arange(K)[None, :]      # [W, K]
    val = (jm >= 0) & (jm < OH)
    jc = np.where(val, jm, 0)
    kwm = np.broadcast_to(2 - np.arange(K)[None, :], (W, K))      # [W, K]

    in_maps = []
    for r0 in ROW0:
        # ---- x tile: [c2=128, slab(5), w, b] bf16
        xr = np.zeros((C, 7, W, B), dtype=np.float32)
        n = min(7, H - r0)
        xr[:, :n] = x[:, :, r0:r0 + n, :].transpose(1, 2, 3, 0)
        top = xr[:, SLABS]                     # [C, 5, W, B]
        bot = xr[:, [s + 1 for s in SLABS]]
        xt = np.concatenate([top, bot], axis=0).astype(_BF16)

        # ---- weight chunks: weight[c, o, i, j, kh, kw], rows i = r0..r0+3
        Wc = weight[:, :, r0:r0 + R]           # [C, O, R, OW, K, K]
        # Bkh[kh][c, o, i, w, t] = W[c,o,i, j(w,t), kh, kw(t)] * valid
        Bkh = []
        for kh in range(K):
            S = Wc[:, :, :, :, kh, :]          # [C, O, R, OW, K(kw)]
            g = S[:, :, :, jc, kwm]            # [C, O, R, W, K(t)]
            g = g * val[None, None, None]
            Bkh.append(g)
        # pair chunks: [c2, i, w, t, o]: top=kh0, bottom=kh1
        pair = np.concatenate([Bkh[0], Bkh[1]], axis=0)   # [128, O, R, W, K]
        pair = pair.transpose(0, 2, 3, 4, 1).astype(_BF16)  # [128, R, W, K, O]
        wpc = np.ascontiguousarray(pair.reshape(B, R * WCH))
        # singles: kh=2 for row i -> Bkh[2][:, :, i]
        # ws0: partitions 0-63 = single(i=1), 64-127 = single(i=0)
        # ws1: partitions 0-63 = single(i=3), 64-127 = single(i=2)
        s = Bkh[2].transpose(0, 2, 3, 4, 1)    # [C, R, W, K, O]
        ws0 = np.concatenate([s[:, 1], s[:, 0]], axis=0)   # [128, W, K, O]
        ws1 = np.concatenate([s[:, 3], s[:, 2]], axis=0)
        wsc = np.stack([ws0, ws1], axis=1).astype(_BF16)   # [128, 2, W, K, O]

        # ---- bias broadcast over b partitions: [b, i, j, o] f32
        bb = bias[:, r0:r0 + R, :].transpose(1, 2, 0)      # [R, OW, O]
        bb = np.broadcast_to(bb.reshape(1, OUT_FREE), (B, OUT_FREE))

        in_maps.append({
            "xt": np.ascontiguousarray(xt.reshape(B, XT_FREE)),
            "wp": wpc,
            "ws": np.ascontiguousarray(wsc.reshape(B, 2 * WCH)),
            "biasb": np.ascontiguousarray(bb, dtype=np.float32),
        })
    return in_maps


def gather_output(results):
    out = np.empty((B, O, OH, OW), dtype=np.float32)
    for k, r0 in enumerate(ROW0):
        co = results[k]["outp"].reshape(B, R, OW, O).astype(np.float32)
        lo = 0 if k < 7 else 2
        out[:, :, r0 + lo:r0 + R, :] = co[:, lo:].transpose(0, 3, 1, 2)
    return out


_NC_CACHE = {}


def kernel(x: np.ndarray, weight: np.ndarray, bias: np.ndarray) -> np.ndarray:
    if "nc" not in _NC_CACHE:
        _NC_CACHE["nc"] = build_nc()
    nc = _NC_CACHE["nc"]
    in_maps = prep_inputs(np.asarray(x), np.asarray(weight), np.asarray(bias))
    res = run_bass_kernel_spmd(nc, in_maps, core_ids=list(range(N_CORES)))
    return gather_output(res.results)


# revision 15
# speedup vs baseline: 2.2443x; 1.0666x over previous
"""Locally-connected layer (3x3, stride 1) on 8 TRN2 NeuronCores — v2.

Math: out[b,o,i,j] = sum_{c,kh,kw} x[b,c,i+kh,j+kw] * W[c,o,i,j,kh,kw] + bias[o,i,j]
  x: [128, 64, 32, 32] f32, W: [64, 64, 30, 30, 3, 3] f32, bias: [64, 30, 30] f32
  out: [128, 64, 30, 30] f32

Sharding: each core owns 4 output rows (cores 6,7 overlap rows so all cores run
an identical program; host keeps rows 28-29 from core 7).

Per-core schedule (data bf16, PSUM f32):
- Contract (c, kh-pair): SBUF x tile packs channels of pixel rows (h, h+1) on
  partitions (0-63, 64-127).  A "pair" matmul contracts kh=(0,1) for output row
  i=h in one 128-deep pass; kh=2 is a 64-deep "single" matmul against one half
  of the same loaded stationary (row-group select via base partition).
- Diagonal weight layout: for stationary x-column w, one matmul covers output
  columns j=w-2..w (kw=2-t), N up to 192.  Matmuls split at PSUM bank edges.
- PSUM: one bank per (row, j-band of 8).  start=True on the first matmul into a
  bank clears its has_written bits; per-element first touch then overwrites.
- Eviction: DVE (psum + bias) -> bf16 out staging; 2 contiguous out DMAs.
  Host upcasts to f32.

Sweep order (h = x slab, pairs (h,h+1)): h=0 pair(i=0); h=1 pair(i=1) +
single(i=0, bottom); h=2 pair(i=2); h=3 singles(i=1 top, i=2 bottom);
h=3 pair(i=3); h=5 single(i=3, top).  Keeps <= 8 PSUM banks live.
"""

import sys

for _p in ("/opt/trn_rl_repo",):
    if _p not in sys.path:
        sys.path.insert(0, _p)

import numpy as np
import ml_dtypes

import concourse.bass as bass
import concourse.tile as tile
from concourse import bacc, mybir
from concourse.bass_utils import run_bass_kernel_spmd

N_CORES = 8
B = 128
C = 64
O = 64
H = 32
W = 32
K = 3
OH = 30
OW = 30
R = 4
ROW0 = [0, 4, 8, 12, 16, 20, 24, 26]

NSLAB = 5                      # x slabs at h = 0,1,2,3,5 (pair = rows (h, h+1))
SLABS = [0, 1, 2, 3, 5]
XT_FREE = NSLAB * W * B        # 20480 elems / partition
WCH = W * K * O                # 6144: one weight chunk (w, t, o)
OUT_FREE = R * OW * O          # 7680

_BF16 = ml_dtypes.bfloat16
STAGGERED = True   # staggered semaphore reset on the timing loop back-edge


def _bank_split(j_lo, j_hi):
    """Split [j_lo, j_hi] into (a, b, bank) within-bank runs."""
    segs = []
    a = j_lo
    while a <= j_hi:
        bk = a // 8
        b = min(j_hi, bk * 8 + 7)
        segs.append((a, b, bk))
        a = b + 1
    return segs


def _segments(w, kind):
    """(a, b, bank, fresh) matmul output runs for stationary column w.

    A pair sweep writes column j=w for the first time (fresh: overwrite path,
    start=True at bank starts) while j=w-2..w-1 accumulate; each matmul's
    region must be homogeneous, so fresh and old are separate matmuls.
    Single sweeps always accumulate (the row's pair sweep ran first)."""
    j_lo, j_hi = max(0, w - 2), min(OH - 1, w)
    if j_lo > j_hi:
        return []
    if kind == "pair":
        segs = []
        if w >= 1:
            segs += [(a, b, bk, False) for (a, b, bk)
                     in _bank_split(j_lo, min(OH - 1, w - 1))]
        if w <= OH - 1:
            segs.append((w, w, w // 8, True))
        return segs
    return [(a, b, bk, False) for (a, b, bk) in _bank_split(j_lo, j_hi)]


def build_nc(repeat: int = 1, mode: str = "full"):
    nc = bacc.Bacc("TRN2", target_bir_lowering=False, debug=False,
                   num_devices=N_CORES)
    xt_ap = nc.dram_tensor("xt", [B, XT_FREE], mybir.dt.bfloat16,
                           kind="ExternalInput").ap()
    wp_ap = nc.dram_tensor("wp", [B, R * WCH], mybir.dt.bfloat16,
                           kind="ExternalInput").ap()
    ws_ap = nc.dram_tensor("ws", [B, 2 * WCH], mybir.dt.bfloat16,
                           kind="ExternalInput").ap()
    bias_ap = nc.dram_tensor("biasb", [B, OUT_FREE], mybir.dt.float32,
                             kind="ExternalInput").ap()
    out_ap = nc.dram_tensor("outp", [B, OUT_FREE], mybir.dt.bfloat16,
                            kind="ExternalOutput").ap()

    with tile.TileContext(nc) as tc:
        with (
            tc.tile_pool(name="xpool", bufs=1) as xpool,
            tc.tile_pool(name="bpool", bufs=1) as bpool,
            tc.tile_pool(name="wppool", bufs=4) as wppool,
            tc.tile_pool(name="wspool", bufs=2) as wspool,
            tc.tile_pool(name="ppool", bufs=8, space="PSUM") as ppool,
            tc.tile_pool(name="opool", bufs=2) as opool,
        ):
            xt_sb = xpool.tile([B, XT_FREE], mybir.dt.bfloat16)
            nc.scalar.dma_start(xt_sb[:, :XT_FREE // 2], xt_ap[:, :XT_FREE // 2])
            nc.scalar.dma_start(xt_sb[:, XT_FREE // 2:], xt_ap[:, XT_FREE // 2:])
            bias_sb = bpool.tile([B, OUT_FREE], mybir.dt.float32)
            nc.scalar.dma_start(bias_sb, bias_ap)
            x4 = xt_sb[:].rearrange("p (h w b) -> p h w b", w=W, b=B)

            FULL, TOP, BOT = (0, B), (0, C), (C, B)

            def body():
                if mode == "empty":
                    return
                wp, ws = {}, {}

                def load_wp(i):
                    t = wppool.tile([B, WCH], mybir.dt.bfloat16, tag="wp")
                    nc.sync.dma_start(t, wp_ap[:, i * WCH:(i + 1) * WCH])
                    wp[i] = t[:].rearrange("p (w t o) -> p w t o", t=K, o=O)

                def load_ws(s):
                    t = wspool.tile([B, WCH], mybir.dt.bfloat16, tag="ws")
                    nc.sync.dma_start(t, ws_ap[:, s * WCH:(s + 1) * WCH])
                    ws[s] = t[:].rearrange("p (w t o) -> p w t o", t=K, o=O)

                load_wp(0)
                load_wp(1)
                load_ws(0)
                load_wp(2)
                load_ws(1)
                load_wp(3)

                ps = {}
                out_sb = opool.tile([B, OUT_FREE], mybir.dt.bfloat16)

                def evict(i, bk):
                    off = i * (OW * O) + bk * 8 * O
                    n = (min(OH, bk * 8 + 8) - bk * 8) * O
                    nc.vector.scalar_tensor_tensor(
                        out_sb[:, off:off + n], ps[(i, bk)][:, :n], 1.0,
                        bias_sb[:, off:off + n],
                        op0=mybir.AluOpType.mult, op1=mybir.AluOpType.add)
                    ps.pop((i, bk))

                def sweep(slab, entries):
                    """entries: list of (kind, i, (lo, hi), w-view); emitted
                    interleaved per w so they share the loaded stationary."""
                    for w in range(W):
                        for (kind, i, (lo, hi), wv) in entries:
                            lhsT = x4[lo:hi, slab, w, :]
                            for (a, b, bk, fresh) in _segments(w, kind):
                                t0 = a - (w - 2)
                                nt = b - a + 1
                                rhs = wv[lo:hi, w, t0:t0 + nt, :]
                                start = fresh and w == 8 * bk
                                if start:
                                    ps[(i, bk)] = ppool.tile(
                                        [B, 512], mybir.dt.float32, tag="ps",
                                        name=f"ps{i}_{bk}")
                                stop = (kind == "single"
                                        and ((bk < 3 and w == 8 * bk + 9)
                                             or (bk == 3 and w == W - 1)))
                                dst = ps[(i, bk)][:, (a - 8 * bk) * O:
                                                  (b + 1 - 8 * bk) * O]
                                nc.tensor.matmul(dst, lhsT, rhs,
                                                 start=start, stop=stop)
                                if stop:
                                    evict(i, bk)

                if mode != "nomm":
                    sweep(0, [("pair", 0, FULL, wp[0])])
                    sweep(1, [("pair", 1, FULL, wp[1]),
                              ("single", 0, BOT, ws[0])])
                    sweep(2, [("pair", 2, FULL, wp[2])])
                    sweep(3, [("single", 1, TOP, ws[0]),
                              ("single", 2, BOT, ws[1])])
                    sweep(3, [("pair", 3, FULL, wp[3])])
                    sweep(4, [("single", 3, TOP, ws[1])])

                nc.scalar.dma_start(out_ap[:, :OUT_FREE // 2],
                                    out_sb[:, :OUT_FREE // 2])
                nc.scalar.dma_start(out_ap[:, OUT_FREE // 2:],
                                    out_sb[:, OUT_FREE // 2:])

            if repeat == 1:
                body()
            else:
                with tc.For_i(0, repeat, 1,
                              hint_engines=(mybir.EngineType.PE,),
                              staggered_reset=STAGGERED):
                    body()

    nc.compile()
    dedup_ldweights(nc)
    return nc


def _ldw_desc(inst):
    """(memref, free_offset, pitch, p0, np, free_ap, dtype) of a LdW, or None."""
    try:
        ap = inst.ins[0]
        pitch, npart = ap.ap[0]
        p0 = ap.bass_ap.base_partition()
        free = tuple(tuple(d) for d in list(ap.ap)[1:])
        return (str(ap.memref), int(ap.offset) - p0 * int(pitch), int(pitch),
                int(p0), int(npart), free, ap.dtype)
    except Exception:
        return None


def dedup_ldweights(nc):
    """Remove InstLdweights that reload PE rows already holding the same data.

    The previous kept LdW loaded partitions [p0, p0+n0) of (memref, offset,
    free pattern); a following LdW whose partition range is a subset with the
    same source is redundant (covers identical APs too).  Conservative: LdWs
    carrying sync waits/updates are kept; any instruction with unknown PE-array
    effect resets tracking.
    """
    removed = 0
    for blk in nc.m.functions[0].blocks:
        insts = list(blk.instructions)
        if not any(type(i).__name__ == "InstLdweights" for i in insts):
            continue
        prev = None
        to_remove = []
        for inst in insts:
            nm = type(inst).__name__
            if nm == "InstLdweights":
                d = _ldw_desc(inst)
                si = inst.sync_info
                clean = not si or (not si.on_wait and not si.on_update)
                if (d is not None and prev is not None and clean
                        and d[0] == prev[0] and d[1] == prev[1]
                        and d[2] == prev[2] and d[5] == prev[5]
                        and d[6] == prev[6]
                        and d[3] >= prev[3]
                        and d[3] + d[4] <= prev[3] + prev[4]):
                    to_remove.append(inst)
                else:
                    prev = d
            elif nm == "InstMatmult":
                pass
            elif nm in ("InstEventSemaphore", "InstNop", "InstTensorLoad",
                        "InstTensorSave"):
                pass
            else:
                prev = None
        for inst in to_remove:
            blk.instructions.remove(inst)
            removed += 1
    return removed


def prep_inputs(x, weight, bias):
    """Host-side shard + relayout + bf16 cast. Returns in_maps for 8 cores."""
    x = np.asarray(x, dtype=np.float32)
    weight = np.asarray(weight, dtype=np.float32)
    bias = np.asarray(bias, dtype=np.float32)

    # (w, t) -> j / kw maps for the diagonal layout
    jm = (np.arange(W)[:, None] - 2) + np.arange(K)[None, :]      # [W, K]
    val = (jm >= 0) & (jm < OH)
    jc = np.where(val, jm, 0)
    kwm = np.broadcast_to(2 - np.arange(K)[None, :], (W, K))      # [W, K]

    in_maps = []
    for r0 in ROW0:
        # ---- x tile: [c2=128, slab(5), w, b] bf16
        xr = np.zeros((C, 7, W, B), dtype=np.float32)
        n = min(7, H - r0)
        xr[:, :n] = x[:, :, r0:r0 + n, :].transpose(1, 2, 3, 0)
        top = xr[:, SLABS]                     # [C, 5, W, B]
        bot = xr[:, [s + 1 for s in SLABS]]
        xt = np.concatenate([top, bot], axis=0).astype(_BF16)

        # ---- weight chunks: weight[c, o, i, j, kh, kw], rows i = r0..r0+3
        Wc = weight[:, :, r0:r0 + R]           # [C, O, R, OW, K, K]
        # Bkh[kh][c, o, i, w, t] = W[c,o,i, j(w,t), kh, kw(t)] * valid
        Bkh = []
        for kh in range(K):
            S = Wc[:, :, :, :, kh, :]          # [C, O, R, OW, K(kw)]
            g = S[:, :, :, jc, kwm]            # [C, O, R, W, K(t)]
            g = g * val[None, None, None]
            Bkh.append(g)
        # pair chunks: [c2, i, w, t, o]: top=kh0, bottom=kh1
        pair = np.concatenate([Bkh[0], Bkh[1]], axis=0)   # [128, O, R, W, K]
        pair = pair.transpose(0, 2, 3, 4, 1).astype(_BF16)  # [128, R, W, K, O]
        wpc = np.ascontiguousarray(pair.reshape(B, R * WCH))
        # singles: kh=2 for row i -> Bkh[2][:, :, i]
        # ws0: partitions 0-63 = single(i=1), 64-127 = single(i=0)
        # ws1: partitions 0-63 = single(i=3), 64-127 = single(i=2)
        s = Bkh[2].transpose(0, 2, 3, 4, 1)    # [C, R, W, K, O]
        ws0 = np.concatenate([s[:, 1], s[:, 0]], axis=0)   # [128, W, K, O]
        ws1 = np.concatenate([s[:, 3], s[:, 2]], axis=0)
        wsc = np.stack([ws0, ws1], axis=1).astype(_BF16)   # [128, 2, W, K, O]

        # ---- bias broadcast over b partitions: [b, i, j, o] f32
        bb = bias[:, r0:r0 + R, :].transpose(1, 2, 0)      # [R, OW, O]
        bb = np.broadcast_to(bb.reshape(1, OUT_FREE), (B, OUT_FREE))

        in_maps.append({
            "xt": np.ascontiguousarray(xt.reshape(B, XT_FREE)),
            "wp": wpc,
            "ws": np.ascontiguousarray(wsc.reshape(B, 2 * WCH)),
            "biasb": np.ascontiguousarray(bb, dtype=np.float32),
        })
    return in_maps


def gather_output(results):
    out = np.empty((B, O, OH, OW), dtype=np.float32)
    for k, r0 in enumerate(ROW0):
        co = results[k]["outp"].reshape(B, R, OW, O).astype(np.float32)
        lo = 0 if k < 7 else 2
        out[:, :, r0 + lo:r0 + R, :] = co[:, lo:].transpose(0, 3, 1, 2)
    return out


_NC_CACHE = {}


def kernel(x: np.ndarray, weight: np.ndarray, bias: np.ndarray) -> np.ndarray:
    if "nc" not in _NC_CACHE:
        _NC_CACHE["nc"] = build_nc()
    nc = _NC_CACHE["nc"]
    in_maps = prep_inputs(np.asarray(x), np.asarray(weight), np.asarray(bias))
    res = run_bass_kernel_spmd(nc, in_maps, core_ids=list(range(N_CORES)))
    return gather_output(res.results)
